# revision 2
# baseline (speedup 1.0000x reference)
"""Trainium2 Bass kernel for MoE top-2 routing (softmax + l_aux + combine weights).

Problem: logits/mask1/mask2 [8192, 64], locations1/2 one-hot [8192, 256].
Outputs: l_aux scalar and combine_weights [8192, 64, 256].

Key structural facts exploited:
  * mask1/mask2 are disjoint one-hot rows and locations are one-hot rows, so
    combine_weights has exactly 2 nonzero elements per token out of 64*256.
    Each nonzero "row" combine_weights[s, e_k, :] equals g_k[s] * loc_k[s, :].
    We therefore only *scatter* 2 rows of 256 floats per token into the
    (pre-zeroed) output via indirect DMA instead of materializing 512 MiB.
  * g1 = num1/(num1+num2) with num_k = sum_e exp(logits - max) * mask_k — the
    softmax denominator cancels, and denom >= 1 so the eps clamp never binds.
  * l_aux only needs column sums of gates (softmax) and mask1; each core emits
    its partial sums and the host finishes the tiny reduction while
    unsharding.

Sharding: tokens split 8 ways (1024 tokens per core); no cross-core
communication is needed on device.
"""

import numpy as np

import concourse.bass as bass
import concourse.bacc as bacc
import concourse.mybir as mybir
from concourse.tile import TileContext
from concourse import bass_utils

S, E, C = 8192, 64, 256
N_CORES = 8
S_LOC = S // N_CORES          # 1024 tokens per core
P = 128                       # partitions
T = S_LOC // P                # 8 column groups per core
F32 = mybir.dt.float32

_CACHE = {}


def _build():
    nc = bacc.Bacc("TRN2", target_bir_lowering=False)

    logits = nc.declare_dram_parameter("logits", [S_LOC, E], F32, isOutput=False)
    m1d = nc.declare_dram_parameter("mask1", [S_LOC, E], F32, isOutput=False)
    m2d = nc.declare_dram_parameter("mask2", [S_LOC, E], F32, isOutput=False)
    l1d = nc.declare_dram_parameter("loc1", [S_LOC, C], F32, isOutput=False)
    l2d = nc.declare_dram_parameter("loc2", [S_LOC, C], F32, isOutput=False)
    cw = nc.declare_dram_parameter("cw", [S_LOC * E, C], F32, isOutput=True)
    partials = nc.declare_dram_parameter("partials", [1, 2 * E], F32, isOutput=True)

    # rowvals[p, t*E + e] = global output row index (s*E + e) of token
    # s = t*P + p, expert e.  Exact in f32 (max 65535 < 2^24).
    rv_np = np.empty((P, T * E), np.float32)
    for t in range(T):
        for p in range(P):
            rv_np[p, t * E:(t + 1) * E] = (t * P + p) * E + np.arange(E)
    rv_dram = nc.inline_tensor(rv_np, "rowvals")

    # batched DRAM views: token s = t*P + p lives at [p, t, :]
    def b3(handle, inner):
        return bass.AP(handle[:].tensor, 0, [[inner, P], [P * inner, T], [1, inner]])

    with TileContext(nc) as tc:
        with (
            tc.tile_pool(name="sbuf", bufs=1) as pool,
            tc.tile_pool(name="psum", bufs=1, space="PSUM") as psum_pool,
        ):
            lt = pool.tile([P, T * E], F32)
            nc.sync.dma_start(lt[:], b3(logits, E))
            m1 = pool.tile([P, T * E], F32)
            nc.sync.dma_start(m1[:], b3(m1d, E))
            m2 = pool.tile([P, T * E], F32)
            nc.sync.dma_start(m2[:], b3(m2d, E))
            l1 = pool.tile([P, T * C], F32)
            nc.sync.dma_start(l1[:], b3(l1d, C))
            l2 = pool.tile([P, T * C], F32)
            nc.sync.dma_start(l2[:], b3(l2d, C))
            rv = pool.tile([P, T * E], F32)
            nc.sync.dma_start(rv[:], rv_dram[:])
            ones = pool.tile([P, 1], F32)
            nc.vector.memset(ones[:], 1.0)

            def v3(tile, inner):  # [P, T*inner] -> [P, T, inner]
                return tile[:].rearrange("p (t i) -> p t i", t=T)

            def bc(tile, inner):  # [P, T] -> [P, T, inner] broadcast
                return tile[:].broadcast_to([P, T, inner])

            lt3, m13, m23 = v3(lt, E), v3(m1, E), v3(m2, E)

            # softmax statistics
            nmax = pool.tile([P, T], F32)
            nc.vector.reduce_max(nmax[:], lt3, axis=mybir.AxisListType.X, negate=True)
            ltc = pool.tile([P, T * E], F32)
            nc.vector.tensor_tensor(v3(ltc, E), lt3, bc(nmax, E), op=mybir.AluOpType.add)
            et = pool.tile([P, T * E], F32)
            nc.scalar.activation(et[:], ltc[:], mybir.ActivationFunctionType.Exp)
            et3 = v3(et, E)
            sume = pool.tile([P, T], F32)
            nc.vector.reduce_sum(sume[:], et3, axis=mybir.AxisListType.X)
            rcp = pool.tile([P, T], F32)
            nc.vector.reciprocal(rcp[:], sume[:])

            # top-2 gate values: g_k = num_k / (num1 + num2)
            p1 = pool.tile([P, T * E], F32)
            nc.vector.tensor_tensor(v3(p1, E), m13, et3, op=mybir.AluOpType.mult)
            num1 = pool.tile([P, T], F32)
            nc.vector.reduce_sum(num1[:], v3(p1, E), axis=mybir.AxisListType.X)
            p2 = pool.tile([P, T * E], F32)
            nc.vector.tensor_tensor(v3(p2, E), m23, et3, op=mybir.AluOpType.mult)
            num2 = pool.tile([P, T], F32)
            nc.vector.reduce_sum(num2[:], v3(p2, E), axis=mybir.AxisListType.X)
            den = pool.tile([P, T], F32)
            nc.vector.tensor_tensor(den[:], num1[:], num2[:], op=mybir.AluOpType.add)
            rden = pool.tile([P, T], F32)
            nc.vector.reciprocal(rden[:], den[:])
            g1 = pool.tile([P, T], F32)
            nc.vector.tensor_tensor(g1[:], num1[:], rden[:], op=mybir.AluOpType.mult)
            g2 = pool.tile([P, T], F32)
            nc.vector.tensor_tensor(g2[:], num2[:], rden[:], op=mybir.AluOpType.mult)

            # full softmax (for l_aux partial sums)
            gates = pool.tile([P, T * E], F32)
            nc.vector.tensor_tensor(v3(gates, E), et3, bc(rcp, E), op=mybir.AluOpType.mult)

            # scatter row indices: ridx_k[p, t] = s*E + argmax_e(mask_k)
            ridx = []
            for mk in (m1, m2):
                q = pool.tile([P, T * E], F32)
                nc.vector.tensor_tensor(v3(q, E), v3(mk, E), v3(rv, E), op=mybir.AluOpType.mult)
                rf = pool.tile([P, T], F32)
                nc.vector.reduce_sum(rf[:], v3(q, E), axis=mybir.AxisListType.X)
                ri = pool.tile([P, T], mybir.dt.int32)
                nc.vector.tensor_copy(ri[:], rf[:])
                ridx.append(ri)

            # scatter row payloads: r_k[p, t, :] = g_k[p, t] * loc_k[p, t, :]
            r1 = pool.tile([P, T * C], F32)
            nc.vector.tensor_tensor(v3(r1, C), v3(l1, C), bc(g1, C), op=mybir.AluOpType.mult)
            r2 = pool.tile([P, T * C], F32)
            nc.vector.tensor_tensor(v3(r2, C), v3(l2, C), bc(g2, C), op=mybir.AluOpType.mult)

            # l_aux partials: column sums over this core's tokens
            me_ps = psum_pool.tile([1, T * E], F32, space="PSUM")
            nc.tensor.matmul(me_ps[:], lhsT=ones[:], rhs=gates[:], start=True, stop=True)
            ce_ps = psum_pool.tile([1, T * E], F32, space="PSUM")
            nc.tensor.matmul(ce_ps[:], lhsT=ones[:], rhs=m1[:], start=True, stop=True)
            part_sb = pool.tile([1, 2 * E], F32)
            for ps, off in ((me_ps, 0), (ce_ps, E)):
                # view [1, (t e)] as [1, e, t] and reduce over t
                pv = bass.AP(ps[:].tensor, ps[:].offset, [ps[:].ap[0], [1, E], [E, T]])
                nc.vector.reduce_sum(part_sb[:1, off:off + E], pv, axis=mybir.AxisListType.X)
            nc.sync.dma_start(partials[:], part_sb[:])

            # scatter the 2*S_LOC nonzero rows into the pre-zeroed output
            for ri, rr in ((ridx[0], r1), (ridx[1], r2)):
                for t in range(T):
                    nc.gpsimd.indirect_dma_start(
                        out=cw[:],
                        out_offset=bass.IndirectOffsetOnAxis(ap=ri[:, t:t + 1], axis=0),
                        in_=rr[:, t * C:(t + 1) * C],
                        in_offset=None,
                    )
    nc.finalize()
    return nc


def _get_nc():
    if "nc" not in _CACHE:
        _CACHE["nc"] = _build()
    return _CACHE["nc"]


def _in_maps(logits, mask1_float, mask2_float, locations1_sc, locations2_sc):
    maps = []
    for c in range(N_CORES):
        sl = slice(c * S_LOC, (c + 1) * S_LOC)
        maps.append({
            "logits": np.ascontiguousarray(logits[sl]),
            "mask1": np.ascontiguousarray(mask1_float[sl]),
            "mask2": np.ascontiguousarray(mask2_float[sl]),
            "loc1": np.ascontiguousarray(locations1_sc[sl]),
            "loc2": np.ascontiguousarray(locations2_sc[sl]),
        })
    return maps


def _install_ntff_shim():
    """The agent image's antenv lacks axon_hooks; provide it so trace=True
    can capture NTFF profiles via the libaxon ctypes path."""
    import sys
    import types

    if "antenv.axon_hooks" in sys.modules:
        return
    try:
        import antenv
        from trn_agent_boot.trn_boot import _ntff_profile_via_ctypes

        mod = types.ModuleType("antenv.axon_hooks")
        hook = _ntff_profile_via_ctypes("/opt/axon/libaxon_pjrt.so")
        mod._hook = hook
        mod.set_axon_ntff_profile_hook = lambda h: setattr(mod, "_hook", h)
        mod.get_axon_ntff_profile_hook = lambda: mod._hook
        sys.modules["antenv.axon_hooks"] = mod
        antenv.axon_hooks = mod
    except Exception:
        pass


def _run(inputs, trace=False, **kwargs):
    if trace:
        _install_ntff_shim()
    nc = _get_nc()
    maps = _in_maps(**{k: np.asarray(v) for k, v in inputs.items()})
    return bass_utils.run_bass_kernel_spmd(
        nc, maps, core_ids=list(range(N_CORES)), trace=trace, **kwargs
    )


def _assemble(results):
    cw = np.concatenate(
        [results[c]["cw"].reshape(S_LOC, E, C) for c in range(N_CORES)], axis=0
    )
    me_sum = np.zeros(E, np.float64)
    ce_sum = np.zeros(E, np.float64)
    for c in range(N_CORES):
        part = results[c]["partials"].reshape(2 * E)
        me_sum += part[:E]
        ce_sum += part[E:]
    l_aux = np.float32(E * np.sum(me_sum * ce_sum) / (S * S))
    return l_aux, cw


def kernel(**inputs):
    res = _run(inputs)
    return _assemble(res.results)
